# revision 24
# baseline (speedup 1.0000x reference)
"""Trainium2 Bass kernel for GaussianKernelLayer.

y[n] = sum_m softmax(coef)[m] * norm * exp(-0.5*|x_n - c_m|^2),
N=500000, M=256, D=4, sigma=1. Data-parallel over 8 cores (x sharded on N).

v5 design (per core, NP=63488 padded rows, 124 chunks of 512):
  - One K=32 fp16 matmul per (chunk, m-half) assembles the full exp
    argument in log2 units directly in PSUM:
      psum[m, n] = log2e*(x.c - 0.5|x|^2) + [log2(w_m*norm)
                   - 0.5*log2e*|c_m|^2 + S]  =: t'   (gauss = 2^t')
    K is padded 16->32 with zero weight rows: the TRN2 HAM clock-gate
    only un-throttles the PE to 2.4 GHz when the 128x128 array is
    near-fully active, and 4 concurrent K=32 row-group tiles qualify
    (4 dense K=16 streams never warm up - measured).
  - Matmuls are issued in QUADS of adjacent instructions that target
    the 4 row-groups (mains: 2 chunks x 2 m-halves) or the 4
    col-groups (ones-reduce over m, K=128) -> each quad executes
    concurrently in ~215 ns warm (measured 209-211 ns/quad).
  - exp split across two engines, alternating by chunk:
      ACT: activation(Exp, scale=ln2) -> exact 2^t' (fp16)
      DVE: Schraudolph in ONE tensor_scalar: uint16(t'*1024+15*1024-C),
           the bit pattern IS fp16(2^t') (piecewise-linear mantissa).
  - Reduce stage is software-pipelined D chunks behind the mains so the
    in-order PE queue never stalls; rhs DMAs land in 4 row-group bands
    of static SBUF buffers (64 partition-lines per group DMA).
  - Host does O(M) prep, fp16 hi/lo splits, and the final 2^-S scale.
"""

import math

import numpy as np

import concourse.bass as bass
import concourse.bacc as bacc_mod
import concourse.mybir as mybir
from concourse.bass_utils import run_bass_kernel_spmd
from concourse.tile import TileContext

N_CORES = 8
N_TOTAL = 500000
PER_CORE = N_TOTAL // N_CORES  # 62500
CHUNK = 512
NCHUNK = 124
NP = CHUNK * NCHUNK  # 63488
M = 256
D = 4
SIGMA = 1.0

F16 = mybir.dt.float16
F32 = mybir.dt.float32
U16 = mybir.dt.uint16

LOG2E = 1.0 / math.log(2.0)
LN2 = math.log(2.0)
SCH_C = 60.0  # Schraudolph shift, tuned on host sim
SCH_BIAS = float(15 * 1024 - SCH_C)

# chunk -> exp engine (1 = ACT exact exp, 0 = DVE Schraudolph);
# pure alternating: every pair = (ACT, DVE) for maximal engine overlap;
# the ACT/DVE rate gap is compensated via evac-copy placement
ACT_PATTERN = (1, 0)

PIPE_D = 10  # reduce stage lags the matmul stage by this many chunks
FILL_FD = 384  # keep-warm filler free-dim
WARMUP_MM = 8
GP = 4  # chunk-PAIRS per rhs DMA group (8 chunks)
NPAIR = NCHUNK // 2  # 62
NGROUP = (NPAIR + GP - 1) // GP  # 16
NRHSBUF = 4

_CACHE = {}


def _build_nc():
    nc = bacc_mod.Bacc()

    # rhs packed per chunk-pair: 64 partition-lines = {A-chunk rows, A rows
    # again, B rows, B rows again}; DMA groups cover pair ranges with a
    # ramp-up schedule so the first exps start as early as possible.
    rhs_d = nc.dram_tensor("rhs", [NPAIR, 64, CHUNK], F16, kind="ExternalInput")
    # full K=32-padded weights for the four row-group bands (zeros included)
    lhsT_d = nc.dram_tensor("lhsT", [128, 128], F16, kind="ExternalInput")
    y_d = nc.dram_tensor("y", [NP], F32, kind="ExternalOutput")

    with TileContext(nc) as tc:
        with (
            tc.tile_pool(name="const", bufs=1) as constp,
            tc.tile_pool(name="cbp", bufs=PIPE_D + 5) as cbp,
            tc.tile_pool(name="ycp", bufs=3) as ycp,
            tc.tile_pool(name="psp", bufs=3, space="PSUM") as psp,
            tc.tile_pool(name="yp", bufs=1, space="PSUM") as yp,
            tc.tile_pool(name="fillp", bufs=1, space="PSUM") as fillp,
        ):
            # --- constants ---
            lhsT_sb = constp.tile([128, 128], F16)
            nc.sync.dma_start(lhsT_sb[:], lhsT_d[:])
            ones_red = constp.tile([128, 32], F16)
            nc.vector.memset(ones_red[:], 1.0)
            scratch = constp.tile([128, CHUNK], F16)
            nc.vector.memset(scratch[:], 0.0)

            # static rhs buffers; odd 16-line bands are zeroed ONCE (they are
            # read by the K=32-padded matmuls against zero weight rows and
            # must not contain NaN junk)
            rhs_bufs = []
            for i in range(NRHSBUF):
                rb = constp.tile([128, GP * CHUNK], F16, name=f"rhsbuf{i}")
                nc.vector.memset(rb[:], 0.0)
                rhs_bufs.append(rb)

            # --- HAM warm-up: serial full-array K=128 matmuls on memset
            # data (the one shape measured to flip the clock-gate to 2.4
            # GHz; once warm it stays warm - re-throttle needs ~3.4us of
            # CONTIGUOUS PE idle, which the steady state never has) ---
            ps_w = psp.tile([128, 2 * CHUNK], F32, tag="ps")
            for i in range(WARMUP_MM):
                nc.tensor.matmul(
                    ps_w[:, CHUNK * (i % 2) : CHUNK * (i % 2 + 1)],
                    scratch[:, 0:128],
                    scratch[:, 0:CHUNK],
                    start=True,
                    stop=True,
                )

            # keep-warm filler: full-array K=128 matmuls on scratch into a
            # dedicated PSUM bank. The HAM clock-gate re-throttles the PE to
            # 1.2 GHz whenever array activity drops for a ~3.4us window; the
            # real work (K=32 row-tiles + 32-col reduces) alone doesn't
            # sustain enough activity. Fillers at quad boundaries absorb
            # would-be idle and keep the 2.4 GHz clock.
            fill_ps = fillp.tile([128, CHUNK], F32)

            def filler(n=1, fd=None):
                for _ in range(n):
                    nc.tensor.matmul(
                        fill_ps[:, 0 : (fd or FILL_FD)],
                        scratch[:, 0:128],
                        scratch[:, 0 : (fd or FILL_FD)],
                        start=True,
                        stop=True,
                    )

            cbs = {}  # chunk k -> cb fp16 AP
            state = {"yps": None, "next_red": 0}

            def reduce_quad(j):
                """ones-reduce for chunks 4j..4j+3, quad-concurrent."""
                yps = yp.tile([128, CHUNK], F32, tag="yps", name=f"yps_{j}")
                state["yps"] = yps
                quad = [cbs.pop(4 * j + q) for q in range(4)]
                for h in range(2):  # half 0 then half 1 (accumulate)
                    for q in range(4):
                        nc.tensor.matmul(
                            yps[32 * q : 32 * q + 32, :],
                            ones_red[:],
                            quad[q][:, h * CHUNK : (h + 1) * CHUNK],
                            start=(h == 0),
                            stop=(h == 1),
                            tile_position=(0, 32 * q),
                        )
                filler(2 if j < 8 else 1)
                yc = ycp.tile([128, CHUNK], F32, tag="yc")
                if j % 3 == 0:
                    nc.vector.tensor_copy(yc[:], yps[:])
                else:
                    nc.scalar.copy(yc[:], yps[:])
                nc.gpsimd.dma_start(
                    y_d[4 * j * CHUNK : (4 * j + 4) * CHUNK].rearrange(
                        "(p f) -> p f", p=4
                    ),
                    yc[0:97:32, :],
                )

            def exp_stage(k, ps, lo):
                """exp of chunk k from psum tile ps columns [lo, lo+1024)."""
                if ACT_PATTERN[k % len(ACT_PATTERN)]:
                    cb = cbp.tile([128, 2 * CHUNK], F16, tag="cb", name=f"cb_{k}")
                    nc.scalar.activation(
                        cb[:],
                        ps[:, lo : lo + 2 * CHUNK],
                        mybir.ActivationFunctionType.Exp,
                        scale=LN2,
                    )
                    cbs[k] = cb[:]
                else:
                    cb = cbp.tile([128, 2 * CHUNK], U16, tag="cb", name=f"cb_{k}")
                    nc.vector.tensor_scalar(
                        cb[:],
                        ps[:, lo : lo + 2 * CHUNK],
                        1024.0,
                        SCH_BIAS,
                        mybir.AluOpType.mult,
                        mybir.AluOpType.add,
                    )
                    cbs[k] = cb[:].bitcast(F16)

            # ramp-up DMA group schedule (in pairs)
            group_of = []
            group_start = []
            group_sz = []
            p0 = 0
            for gs in (1, 1, 2) + (GP,) * 100:
                if p0 >= NPAIR:
                    break
                gs = min(gs, NPAIR - p0)
                group_start.append(p0)
                group_sz.append(gs)
                group_of.extend([len(group_sz) - 1] * gs)
                p0 += gs

            for p in range(NPAIR):
                gi = group_of[p]
                jj = p - group_start[gi]
                if jj == 0:
                    gsz = group_sz[gi]
                    rhs_t = rhs_bufs[gi % NRHSBUF]
                    for b, eng in (
                        (0, nc.sync),
                        (1, nc.gpsimd),
                        (2, nc.sync),
                        (3, nc.gpsimd),
                    ):
                        eng.dma_start(
                            rhs_t[32 * b : 32 * b + 16, 0 : gsz * CHUNK],
                            rhs_d[
                                group_start[gi] : group_start[gi] + gsz,
                                16 * b : 16 * b + 16,
                                :,
                            ].rearrange("j p f -> p j f"),
                        )
                col = jj * CHUNK
                kA, kB = 2 * p, 2 * p + 1
                psA = psp.tile([128, 2 * CHUNK], F32, tag="ps", name=f"psA_{p}")
                psB = psp.tile([128, 2 * CHUNK], F32, tag="ps", name=f"psB_{p}")
                # quad: (A,h0)->band0, (A,h1)->band32, (B,h0)->band64,
                # (B,h1)->band96 -- concurrent row-group tiles
                for b, ps, lo in (
                    (0, psA, 0),
                    (32, psA, CHUNK),
                    (64, psB, 0),
                    (96, psB, CHUNK),
                ):
                    nc.tensor.matmul(
                        ps[:, lo : lo + CHUNK],
                        lhsT_sb[b : b + 32, :],
                        rhs_t[b : b + 32, col : col + CHUNK],
                        start=True,
                        stop=True,
                        tile_position=(b, 0),
                    )

                filler(2 if p < 8 else 1)
                exp_stage(kA, psA, 0)
                exp_stage(kB, psB, 0)

                while (
                    state["next_red"] * 4 + 3 <= kB - PIPE_D
                    and state["next_red"] * 4 + 3 < NCHUNK
                ):
                    reduce_quad(state["next_red"])
                    state["next_red"] += 1

            while state["next_red"] < NCHUNK // 4:
                reduce_quad(state["next_red"])
                state["next_red"] += 1
    nc.compile()
    return nc


def _host_prep(x, centers, coefficients):
    """O(M) center prep + per-core x layout, all in log2 units."""
    x = np.ascontiguousarray(np.asarray(x, dtype=np.float32))
    centers = np.asarray(centers, dtype=np.float32)
    coefficients = np.asarray(coefficients, dtype=np.float32)

    norm_const = np.float32(1.0 / ((2.0 * math.pi) ** (D / 2) * SIGMA**D))
    e = np.exp(coefficients - coefficients.max())
    w = (e / e.sum()).astype(np.float32)

    s = np.float32(math.sqrt(LOG2E))
    b = centers.T * s  # [4, 256]
    b_hi = b.astype(np.float16)
    b_lo = (b - b_hi.astype(np.float32)).astype(np.float16)

    g_raw = (
        np.log2(w * norm_const) - 0.5 * LOG2E * (centers**2).sum(axis=1)
    ).astype(np.float32)
    S = np.float32(math.floor(12.0 - np.log2(w * norm_const).max()))
    g = g_raw + S
    g_hi = g.astype(np.float16)
    g_lo = (g - g_hi.astype(np.float32)).astype(np.float16)

    halfw = np.zeros((2, 16, 128), dtype=np.float16)
    for h in range(2):
        sl = slice(128 * h, 128 * (h + 1))
        halfw[h, 0:4] = b_hi[:, sl]
        halfw[h, 4:8] = b_hi[:, sl]
        halfw[h, 8:12] = b_lo[:, sl]
        halfw[h, 12] = 1.0
        halfw[h, 13] = 1.0
        halfw[h, 14] = g_hi[sl]
        halfw[h, 15] = g_lo[sl]
    lhsT = np.zeros((128, 128), dtype=np.float16)
    lhsT[0:16] = halfw[0]
    lhsT[32:48] = halfw[1]
    lhsT[64:80] = halfw[0]
    lhsT[96:112] = halfw[1]

    in_maps = []
    for i in range(N_CORES):
        xs = x[i * PER_CORE : (i + 1) * PER_CORE]
        xp = np.zeros((NP, D), dtype=np.float32)
        xp[:PER_CORE] = xs
        a = xp * s
        a_hi = a.astype(np.float16)
        a_lo = (a - a_hi.astype(np.float32)).astype(np.float16)
        hbias = (-0.5 * LOG2E * (xp**2).sum(axis=1)).astype(np.float32)
        h_hi = hbias.astype(np.float16)
        h_lo = (hbias - h_hi.astype(np.float32)).astype(np.float16)
        rows = np.empty((16, NP), dtype=np.float16)
        rows[0:4] = a_hi.T
        rows[4:8] = a_lo.T
        rows[8:12] = a_hi.T
        rows[12] = h_hi
        rows[13] = h_lo
        rows[14] = 1.0
        rows[15] = 1.0
        # pack per pair: 64 lines = {A rows, A rows, B rows, B rows}
        rc = rows.reshape(16, NCHUNK, CHUNK).transpose(1, 0, 2)  # [124,16,512]
        rp = rc.reshape(NPAIR, 2, 16, CHUNK)
        rhs = np.empty((NPAIR, 64, CHUNK), dtype=np.float16)
        rhs[:, 0:16] = rp[:, 0]
        rhs[:, 16:32] = rp[:, 0]
        rhs[:, 32:48] = rp[:, 1]
        rhs[:, 48:64] = rp[:, 1]
        in_maps.append({"rhs": np.ascontiguousarray(rhs), "lhsT": lhsT.copy()})
    return in_maps, float(S)


last_result = None


def kernel(x, centers, coefficients):
    global last_result
    if "nc" not in _CACHE:
        _CACHE["nc"] = _build_nc()
    nc = _CACHE["nc"]
    in_maps, S = _host_prep(x, centers, coefficients)
    res = run_bass_kernel_spmd(nc, in_maps, core_ids=list(range(N_CORES)))
    last_result = res
    y = np.concatenate([r["y"][:PER_CORE] for r in res.results])
    return (y * np.float32(2.0 ** (-S))).astype(np.float32)


# revision 25
# speedup vs baseline: 1.1824x; 1.1824x over previous
"""Trainium2 Bass kernel for GaussianKernelLayer.

y[n] = sum_m softmax(coef)[m] * norm * exp(-0.5*|x_n - c_m|^2),
N=500000, M=256, D=4, sigma=1. Data-parallel over 8 cores (x sharded on N).

v5 design (per core, NP=63488 padded rows, 124 chunks of 512):
  - One K=32 fp16 matmul per (chunk, m-half) assembles the full exp
    argument in log2 units directly in PSUM:
      psum[m, n] = log2e*(x.c - 0.5|x|^2) + [log2(w_m*norm)
                   - 0.5*log2e*|c_m|^2 + S]  =: t'   (gauss = 2^t')
    K is padded 16->32 with zero weight rows: the TRN2 HAM clock-gate
    only un-throttles the PE to 2.4 GHz when the 128x128 array is
    near-fully active, and 4 concurrent K=32 row-group tiles qualify
    (4 dense K=16 streams never warm up - measured).
  - Matmuls are issued in QUADS of adjacent instructions that target
    the 4 row-groups (mains: 2 chunks x 2 m-halves) or the 4
    col-groups (ones-reduce over m, K=128) -> each quad executes
    concurrently in ~215 ns warm (measured 209-211 ns/quad).
  - exp split across two engines, alternating by chunk:
      ACT: activation(Exp, scale=ln2) -> exact 2^t' (fp16)
      DVE: Schraudolph in ONE tensor_scalar: uint16(t'*1024+15*1024-C),
           the bit pattern IS fp16(2^t') (piecewise-linear mantissa).
  - Reduce stage is software-pipelined D chunks behind the mains so the
    in-order PE queue never stalls; rhs DMAs land in 4 row-group bands
    of static SBUF buffers (64 partition-lines per group DMA).
  - Host does O(M) prep, fp16 hi/lo splits, and the final 2^-S scale.
"""

import math

import numpy as np

import concourse.bass as bass
import concourse.bacc as bacc_mod
import concourse.mybir as mybir
from concourse.bass_utils import run_bass_kernel_spmd
from concourse.tile import TileContext

N_CORES = 8
N_TOTAL = 500000
PER_CORE = N_TOTAL // N_CORES  # 62500
CHUNK = 512
NCHUNK = 124
NP = CHUNK * NCHUNK  # 63488
M = 256
D = 4
SIGMA = 1.0

F16 = mybir.dt.float16
F32 = mybir.dt.float32
U16 = mybir.dt.uint16

LOG2E = 1.0 / math.log(2.0)
LN2 = math.log(2.0)
SCH_C = 60.0  # Schraudolph shift, tuned on host sim
SCH_BIAS = float(15 * 1024 - SCH_C)

# chunk -> exp engine (1 = ACT exact exp, 0 = DVE Schraudolph);
# pure alternating: every pair = (ACT, DVE) for maximal engine overlap;
# the ACT/DVE rate gap is compensated via evac-copy placement
ACT_PATTERN = (1, 0)

PIPE_D = 10  # reduce stage lags the matmul stage by this many chunks
FILL_FD = 384  # keep-warm filler free-dim
WARMUP_MM = 8
GP = 4  # chunk-PAIRS per rhs DMA group (8 chunks)
NPAIR = NCHUNK // 2  # 62
NGROUP = (NPAIR + GP - 1) // GP  # 16
NRHSBUF = 4

_CACHE = {}


def _build_nc():
    nc = bacc_mod.Bacc()

    # rhs packed per group of GP chunk-pairs: 64 partition-lines = {A-chunk
    # rows, A rows again, B rows, B rows again} x (pair cols side by side).
    # First two groups are single pairs so the exp pipeline starts early.
    rhs_d = nc.dram_tensor("rhs", [NGROUP + 2, 64, GP * CHUNK], F16, kind="ExternalInput")
    # full K=32-padded weights for the four row-group bands (zeros included)
    lhsT_d = nc.dram_tensor("lhsT", [128, 128], F16, kind="ExternalInput")
    y_d = nc.dram_tensor("y", [NP], F32, kind="ExternalOutput")

    with TileContext(nc) as tc:
        with (
            tc.tile_pool(name="const", bufs=1) as constp,
            tc.tile_pool(name="cbp", bufs=PIPE_D + 5) as cbp,
            tc.tile_pool(name="ycp", bufs=3) as ycp,
            tc.tile_pool(name="psp", bufs=3, space="PSUM") as psp,
            tc.tile_pool(name="yp", bufs=1, space="PSUM") as yp,
            tc.tile_pool(name="fillp", bufs=1, space="PSUM") as fillp,
        ):
            # --- constants ---
            lhsT_sb = constp.tile([128, 128], F16)
            nc.sync.dma_start(lhsT_sb[:], lhsT_d[:])
            ones_red = constp.tile([128, 32], F16)
            nc.vector.memset(ones_red[:], 1.0)
            scratch = constp.tile([128, CHUNK], F16)
            nc.vector.memset(scratch[:], 0.0)

            # static rhs buffers; odd 16-line bands are zeroed ONCE (they are
            # read by the K=32-padded matmuls against zero weight rows and
            # must not contain NaN junk)
            rhs_bufs = []
            for i in range(NRHSBUF):
                rb = constp.tile([128, GP * CHUNK], F16, name=f"rhsbuf{i}")
                nc.vector.memset(rb[:], 0.0)
                rhs_bufs.append(rb)

            # --- HAM warm-up: serial full-array K=128 matmuls on memset
            # data (the one shape measured to flip the clock-gate to 2.4
            # GHz; once warm it stays warm - re-throttle needs ~3.4us of
            # CONTIGUOUS PE idle, which the steady state never has) ---
            ps_w = psp.tile([128, 2 * CHUNK], F32, tag="ps")
            for i in range(WARMUP_MM):
                nc.tensor.matmul(
                    ps_w[:, CHUNK * (i % 2) : CHUNK * (i % 2 + 1)],
                    scratch[:, 0:128],
                    scratch[:, 0:CHUNK],
                    start=True,
                    stop=True,
                )

            # keep-warm filler: full-array K=128 matmuls on scratch into a
            # dedicated PSUM bank. The HAM clock-gate re-throttles the PE to
            # 1.2 GHz whenever array activity drops for a ~3.4us window; the
            # real work (K=32 row-tiles + 32-col reduces) alone doesn't
            # sustain enough activity. Fillers at quad boundaries absorb
            # would-be idle and keep the 2.4 GHz clock.
            fill_ps = fillp.tile([128, CHUNK], F32)

            def filler(n=1, fd=None):
                for _ in range(n):
                    nc.tensor.matmul(
                        fill_ps[:, 0 : (fd or FILL_FD)],
                        scratch[:, 0:128],
                        scratch[:, 0 : (fd or FILL_FD)],
                        start=True,
                        stop=True,
                    )

            cbs = {}  # chunk k -> cb fp16 AP
            state = {"yps": None, "next_red": 0}

            def reduce_quad(j):
                """ones-reduce for chunks 4j..4j+3, quad-concurrent."""
                yps = yp.tile([128, CHUNK], F32, tag="yps", name=f"yps_{j}")
                state["yps"] = yps
                quad = [cbs.pop(4 * j + q) for q in range(4)]
                for h in range(2):  # half 0 then half 1 (accumulate)
                    for q in range(4):
                        nc.tensor.matmul(
                            yps[32 * q : 32 * q + 32, :],
                            ones_red[:],
                            quad[q][:, h * CHUNK : (h + 1) * CHUNK],
                            start=(h == 0),
                            stop=(h == 1),
                            tile_position=(0, 32 * q),
                        )
                filler(2 if j < 8 else 1)
                yc = ycp.tile([128, CHUNK], F32, tag="yc")
                if j % 3 == 0:
                    nc.vector.tensor_copy(yc[:], yps[:])
                else:
                    nc.scalar.copy(yc[:], yps[:])
                nc.gpsimd.dma_start(
                    y_d[4 * j * CHUNK : (4 * j + 4) * CHUNK].rearrange(
                        "(p f) -> p f", p=4
                    ),
                    yc[0:97:32, :],
                )

            def exp_stage(k, ps, lo):
                """exp of chunk k from psum tile ps columns [lo, lo+1024)."""
                if ACT_PATTERN[k % len(ACT_PATTERN)]:
                    cb = cbp.tile([128, 2 * CHUNK], F16, tag="cb", name=f"cb_{k}")
                    nc.scalar.activation(
                        cb[:],
                        ps[:, lo : lo + 2 * CHUNK],
                        mybir.ActivationFunctionType.Exp,
                        scale=LN2,
                    )
                    cbs[k] = cb[:]
                else:
                    cb = cbp.tile([128, 2 * CHUNK], U16, tag="cb", name=f"cb_{k}")
                    nc.vector.tensor_scalar(
                        cb[:],
                        ps[:, lo : lo + 2 * CHUNK],
                        1024.0,
                        SCH_BIAS,
                        mybir.AluOpType.mult,
                        mybir.AluOpType.add,
                    )
                    cbs[k] = cb[:].bitcast(F16)

            # group schedule (in pairs): two single-pair ramp-up groups,
            # then full GP-pair groups
            group_start = []
            group_sz = []
            p0 = 0
            for gs in (1, 1) + (GP,) * 100:
                if p0 >= NPAIR:
                    break
                gs = min(gs, NPAIR - p0)
                group_start.append(p0)
                group_sz.append(gs)
                p0 += gs
            group_of = []
            for gi2, gs in enumerate(group_sz):
                group_of.extend([gi2] * gs)

            for p in range(NPAIR):
                gi = group_of[p]
                jj = p - group_start[gi]
                if jj == 0:
                    gsz = group_sz[gi]
                    rhs_t = rhs_bufs[gi % NRHSBUF]
                    for b, eng in (
                        (0, nc.sync),
                        (1, nc.gpsimd),
                        (2, nc.sync),
                        (3, nc.gpsimd),
                    ):
                        eng.dma_start(
                            rhs_t[32 * b : 32 * b + 16, 0 : gsz * CHUNK],
                            rhs_d[gi, 16 * b : 16 * b + 16, 0 : gsz * CHUNK],
                        )
                col = jj * CHUNK
                kA, kB = 2 * p, 2 * p + 1
                psA = psp.tile([128, 2 * CHUNK], F32, tag="ps", name=f"psA_{p}")
                psB = psp.tile([128, 2 * CHUNK], F32, tag="ps", name=f"psB_{p}")
                # quad: (A,h0)->band0, (A,h1)->band32, (B,h0)->band64,
                # (B,h1)->band96 -- concurrent row-group tiles
                for b, ps, lo in (
                    (0, psA, 0),
                    (32, psA, CHUNK),
                    (64, psB, 0),
                    (96, psB, CHUNK),
                ):
                    nc.tensor.matmul(
                        ps[:, lo : lo + CHUNK],
                        lhsT_sb[b : b + 32, :],
                        rhs_t[b : b + 32, col : col + CHUNK],
                        start=True,
                        stop=True,
                        tile_position=(b, 0),
                    )

                filler(2 if p < 8 else 1)
                exp_stage(kA, psA, 0)
                exp_stage(kB, psB, 0)

                while (
                    state["next_red"] * 4 + 3 <= kB - PIPE_D
                    and state["next_red"] * 4 + 3 < NCHUNK
                ):
                    reduce_quad(state["next_red"])
                    state["next_red"] += 1

            while state["next_red"] < NCHUNK // 4:
                reduce_quad(state["next_red"])
                state["next_red"] += 1
    nc.compile()
    return nc


def _host_prep(x, centers, coefficients):
    """O(M) center prep + per-core x layout, all in log2 units."""
    x = np.ascontiguousarray(np.asarray(x, dtype=np.float32))
    centers = np.asarray(centers, dtype=np.float32)
    coefficients = np.asarray(coefficients, dtype=np.float32)

    norm_const = np.float32(1.0 / ((2.0 * math.pi) ** (D / 2) * SIGMA**D))
    e = np.exp(coefficients - coefficients.max())
    w = (e / e.sum()).astype(np.float32)

    s = np.float32(math.sqrt(LOG2E))
    b = centers.T * s  # [4, 256]
    b_hi = b.astype(np.float16)
    b_lo = (b - b_hi.astype(np.float32)).astype(np.float16)

    g_raw = (
        np.log2(w * norm_const) - 0.5 * LOG2E * (centers**2).sum(axis=1)
    ).astype(np.float32)
    S = np.float32(math.floor(12.0 - np.log2(w * norm_const).max()))
    g = g_raw + S
    g_hi = g.astype(np.float16)
    g_lo = (g - g_hi.astype(np.float32)).astype(np.float16)

    halfw = np.zeros((2, 16, 128), dtype=np.float16)
    for h in range(2):
        sl = slice(128 * h, 128 * (h + 1))
        halfw[h, 0:4] = b_hi[:, sl]
        halfw[h, 4:8] = b_hi[:, sl]
        halfw[h, 8:12] = b_lo[:, sl]
        halfw[h, 12] = 1.0
        halfw[h, 13] = 1.0
        halfw[h, 14] = g_hi[sl]
        halfw[h, 15] = g_lo[sl]
    lhsT = np.zeros((128, 128), dtype=np.float16)
    lhsT[0:16] = halfw[0]
    lhsT[32:48] = halfw[1]
    lhsT[64:80] = halfw[0]
    lhsT[96:112] = halfw[1]

    in_maps = []
    for i in range(N_CORES):
        xs = x[i * PER_CORE : (i + 1) * PER_CORE]
        xp = np.zeros((NP, D), dtype=np.float32)
        xp[:PER_CORE] = xs
        a = xp * s
        a_hi = a.astype(np.float16)
        a_lo = (a - a_hi.astype(np.float32)).astype(np.float16)
        hbias = (-0.5 * LOG2E * (xp**2).sum(axis=1)).astype(np.float32)
        h_hi = hbias.astype(np.float16)
        h_lo = (hbias - h_hi.astype(np.float32)).astype(np.float16)
        rows = np.empty((16, NP), dtype=np.float16)
        rows[0:4] = a_hi.T
        rows[4:8] = a_lo.T
        rows[8:12] = a_hi.T
        rows[12] = h_hi
        rows[13] = h_lo
        rows[14] = 1.0
        rows[15] = 1.0
        # pack per group: 64 lines = {A rows, A rows, B rows, B rows},
        # pair columns side by side; groups = (1, 1, GP, GP, ...) pairs
        rc = rows.reshape(16, NCHUNK, CHUNK).transpose(1, 0, 2)  # [124,16,512]
        rp = rc.reshape(NPAIR, 2, 16, CHUNK)
        gs_list = []
        p0 = 0
        for gs in (1, 1) + (GP,) * 100:
            if p0 >= NPAIR:
                break
            gs = min(gs, NPAIR - p0)
            gs_list.append((p0, gs))
            p0 += gs
        rhs = np.zeros((NGROUP + 2, 64, GP * CHUNK), dtype=np.float16)
        for gi, (p0, gsz) in enumerate(gs_list):
            for j in range(gsz):
                cs = slice(j * CHUNK, (j + 1) * CHUNK)
                rhs[gi, 0:16, cs] = rp[p0 + j, 0]
                rhs[gi, 16:32, cs] = rp[p0 + j, 0]
                rhs[gi, 32:48, cs] = rp[p0 + j, 1]
                rhs[gi, 48:64, cs] = rp[p0 + j, 1]
        in_maps.append({"rhs": rhs, "lhsT": lhsT.copy()})
    return in_maps, float(S)


last_result = None


def kernel(x, centers, coefficients):
    global last_result
    if "nc" not in _CACHE:
        _CACHE["nc"] = _build_nc()
    nc = _CACHE["nc"]
    in_maps, S = _host_prep(x, centers, coefficients)
    res = run_bass_kernel_spmd(nc, in_maps, core_ids=list(range(N_CORES)))
    last_result = res
    y = np.concatenate([r["y"][:PER_CORE] for r in res.results])
    return (y * np.float32(2.0 ** (-S))).astype(np.float32)


# revision 26
# speedup vs baseline: 1.3370x; 1.1307x over previous
"""Trainium2 Bass kernel for GaussianKernelLayer.

y[n] = sum_m softmax(coef)[m] * norm * exp(-0.5*|x_n - c_m|^2),
N=500000, M=256, D=4, sigma=1. Data-parallel over 8 cores (x sharded on N).

v5 design (per core, NP=63488 padded rows, 124 chunks of 512):
  - One K=32 fp16 matmul per (chunk, m-half) assembles the full exp
    argument in log2 units directly in PSUM:
      psum[m, n] = log2e*(x.c - 0.5|x|^2) + [log2(w_m*norm)
                   - 0.5*log2e*|c_m|^2 + S]  =: t'   (gauss = 2^t')
    K is padded 16->32 with zero weight rows: the TRN2 HAM clock-gate
    only un-throttles the PE to 2.4 GHz when the 128x128 array is
    near-fully active, and 4 concurrent K=32 row-group tiles qualify
    (4 dense K=16 streams never warm up - measured).
  - Matmuls are issued in QUADS of adjacent instructions that target
    the 4 row-groups (mains: 2 chunks x 2 m-halves) or the 4
    col-groups (ones-reduce over m, K=128) -> each quad executes
    concurrently in ~215 ns warm (measured 209-211 ns/quad).
  - exp split across two engines, alternating by chunk:
      ACT: activation(Exp, scale=ln2) -> exact 2^t' (fp16)
      DVE: Schraudolph in ONE tensor_scalar: uint16(t'*1024+15*1024-C),
           the bit pattern IS fp16(2^t') (piecewise-linear mantissa).
  - Reduce stage is software-pipelined D chunks behind the mains so the
    in-order PE queue never stalls; rhs DMAs land in 4 row-group bands
    of static SBUF buffers (64 partition-lines per group DMA).
  - Host does O(M) prep, fp16 hi/lo splits, and the final 2^-S scale.
"""

import math

import numpy as np

import concourse.bass as bass
import concourse.bacc as bacc_mod
import concourse.mybir as mybir
from concourse.bass_utils import run_bass_kernel_spmd
from concourse.tile import TileContext

N_CORES = 8
N_TOTAL = 500000
PER_CORE = N_TOTAL // N_CORES  # 62500
CHUNK = 512
NCHUNK = 124
NP = CHUNK * NCHUNK  # 63488
M = 256
D = 4
SIGMA = 1.0

F16 = mybir.dt.float16
F32 = mybir.dt.float32
U16 = mybir.dt.uint16

LOG2E = 1.0 / math.log(2.0)
LN2 = math.log(2.0)
SCH_C = 60.0  # Schraudolph shift, tuned on host sim
SCH_BIAS = float(15 * 1024 - SCH_C)

# chunk -> exp engine (1 = ACT exact exp, 0 = DVE Schraudolph);
# pure alternating: every pair = (ACT, DVE) for maximal engine overlap;
# the ACT/DVE rate gap is compensated via evac-copy placement
ACT_PATTERN = (1, 0)

PIPE_D = 10  # reduce stage lags the matmul stage by this many chunks
FILL_FD = 384  # keep-warm filler free-dim
WARMUP_MM = 14
GP = 4  # chunk-PAIRS per rhs DMA group (8 chunks)
NPAIR = NCHUNK // 2  # 62
NGROUP = (NPAIR + GP - 1) // GP  # 16
NRHSBUF = 4

_CACHE = {}


def _build_nc():
    nc = bacc_mod.Bacc()

    # rhs packed per group of GP chunk-pairs: 64 partition-lines = {A-chunk
    # rows, A rows again, B rows, B rows again} x (pair cols side by side).
    # First two groups are single pairs so the exp pipeline starts early.
    rhs_d = nc.dram_tensor("rhs", [NGROUP + 2, 64, GP * CHUNK], F16, kind="ExternalInput")
    # full K=32-padded weights for the four row-group bands (zeros included)
    lhsT_d = nc.dram_tensor("lhsT", [128, 128], F16, kind="ExternalInput")
    y_d = nc.dram_tensor("y", [NP], F32, kind="ExternalOutput")

    with TileContext(nc) as tc:
        with (
            tc.tile_pool(name="const", bufs=1) as constp,
            tc.tile_pool(name="cbp", bufs=PIPE_D + 5) as cbp,
            tc.tile_pool(name="ycp", bufs=3) as ycp,
            tc.tile_pool(name="psp", bufs=3, space="PSUM") as psp,
            tc.tile_pool(name="yp", bufs=1, space="PSUM") as yp,
            tc.tile_pool(name="fillp", bufs=1, space="PSUM") as fillp,
        ):
            # --- constants ---
            lhsT_sb = constp.tile([128, 128], F16)
            nc.sync.dma_start(lhsT_sb[:], lhsT_d[:])
            ones_red = constp.tile([128, 32], F16)
            nc.vector.memset(ones_red[:], 1.0)
            scratch = constp.tile([128, CHUNK], F16)
            nc.vector.memset(scratch[:], 0.0)

            # static rhs buffers; odd 16-line bands are zeroed ONCE (they are
            # read by the K=32-padded matmuls against zero weight rows and
            # must not contain NaN junk)
            rhs_bufs = []
            for i in range(NRHSBUF):
                rb = constp.tile([128, GP * CHUNK], F16, name=f"rhsbuf{i}")
                nc.vector.memset(rb[:], 0.0)
                rhs_bufs.append(rb)

            # --- HAM warm-up: serial full-array K=128 matmuls on memset
            # data (the one shape measured to flip the clock-gate to 2.4
            # GHz; once warm it stays warm - re-throttle needs ~3.4us of
            # CONTIGUOUS PE idle, which the steady state never has) ---
            ps_w = psp.tile([128, 2 * CHUNK], F32, tag="ps")
            for i in range(WARMUP_MM):
                nc.tensor.matmul(
                    ps_w[:, CHUNK * (i % 2) : CHUNK * (i % 2 + 1)],
                    scratch[:, 0:128],
                    scratch[:, 0:CHUNK],
                    start=True,
                    stop=True,
                )

            # keep-warm filler: full-array K=128 matmuls on scratch into a
            # dedicated PSUM bank. The HAM clock-gate re-throttles the PE to
            # 1.2 GHz whenever array activity drops for a ~3.4us window; the
            # real work (K=32 row-tiles + 32-col reduces) alone doesn't
            # sustain enough activity. Fillers at quad boundaries absorb
            # would-be idle and keep the 2.4 GHz clock.
            fill_ps = fillp.tile([128, CHUNK], F32)

            def filler(n=1, fd=None):
                for _ in range(n):
                    nc.tensor.matmul(
                        fill_ps[:, 0 : (fd or FILL_FD)],
                        scratch[:, 0:128],
                        scratch[:, 0 : (fd or FILL_FD)],
                        start=True,
                        stop=True,
                    )

            cbs = {}  # chunk k -> cb fp16 AP
            state = {"yps": None, "next_red": 0}

            def reduce_quad(j):
                """ones-reduce for chunks 4j..4j+3, quad-concurrent."""
                yps = yp.tile([128, CHUNK], F32, tag="yps", name=f"yps_{j}")
                state["yps"] = yps
                quad = [cbs.pop(4 * j + q) for q in range(4)]
                for h in range(2):  # half 0 then half 1 (accumulate)
                    for q in range(4):
                        nc.tensor.matmul(
                            yps[32 * q : 32 * q + 32, :],
                            ones_red[:],
                            quad[q][:, h * CHUNK : (h + 1) * CHUNK],
                            start=(h == 0),
                            stop=(h == 1),
                            tile_position=(0, 32 * q),
                        )
                filler(2 if j < 8 else 1)
                yc = ycp.tile([128, CHUNK], F32, tag="yc")
                if j % 3 == 0:
                    nc.vector.tensor_copy(yc[:], yps[:])
                else:
                    nc.scalar.copy(yc[:], yps[:])
                nc.gpsimd.dma_start(
                    y_d[4 * j * CHUNK : (4 * j + 4) * CHUNK].rearrange(
                        "(p f) -> p f", p=4
                    ),
                    yc[0:97:32, :],
                )

            def exp_stage(k, ps, lo):
                """exp of chunk k from psum tile ps columns [lo, lo+1024)."""
                if ACT_PATTERN[k % len(ACT_PATTERN)]:
                    cb = cbp.tile([128, 2 * CHUNK], F16, tag="cb", name=f"cb_{k}")
                    nc.scalar.activation(
                        cb[:],
                        ps[:, lo : lo + 2 * CHUNK],
                        mybir.ActivationFunctionType.Exp,
                        scale=LN2,
                    )
                    cbs[k] = cb[:]
                else:
                    cb = cbp.tile([128, 2 * CHUNK], U16, tag="cb", name=f"cb_{k}")
                    nc.vector.tensor_scalar(
                        cb[:],
                        ps[:, lo : lo + 2 * CHUNK],
                        1024.0,
                        SCH_BIAS,
                        mybir.AluOpType.mult,
                        mybir.AluOpType.add,
                    )
                    cbs[k] = cb[:].bitcast(F16)

            # group schedule (in pairs): two single-pair ramp-up groups,
            # then full GP-pair groups
            group_start = []
            group_sz = []
            p0 = 0
            for gs in (GP,) * 100:
                if p0 >= NPAIR:
                    break
                gs = min(gs, NPAIR - p0)
                group_start.append(p0)
                group_sz.append(gs)
                p0 += gs
            group_of = []
            for gi2, gs in enumerate(group_sz):
                group_of.extend([gi2] * gs)

            for p in range(NPAIR):
                gi = group_of[p]
                jj = p - group_start[gi]
                if jj == 0:
                    gsz = group_sz[gi]
                    rhs_t = rhs_bufs[gi % NRHSBUF]
                    for b, eng in (
                        (0, nc.sync),
                        (1, nc.gpsimd),
                        (2, nc.sync),
                        (3, nc.gpsimd),
                    ):
                        eng.dma_start(
                            rhs_t[32 * b : 32 * b + 16, 0 : gsz * CHUNK],
                            rhs_d[gi, 16 * b : 16 * b + 16, 0 : gsz * CHUNK],
                        )
                col = jj * CHUNK
                kA, kB = 2 * p, 2 * p + 1
                psA = psp.tile([128, 2 * CHUNK], F32, tag="ps", name=f"psA_{p}")
                psB = psp.tile([128, 2 * CHUNK], F32, tag="ps", name=f"psB_{p}")
                # quad: (A,h0)->band0, (A,h1)->band32, (B,h0)->band64,
                # (B,h1)->band96 -- concurrent row-group tiles
                for b, ps, lo in (
                    (0, psA, 0),
                    (32, psA, CHUNK),
                    (64, psB, 0),
                    (96, psB, CHUNK),
                ):
                    nc.tensor.matmul(
                        ps[:, lo : lo + CHUNK],
                        lhsT_sb[b : b + 32, :],
                        rhs_t[b : b + 32, col : col + CHUNK],
                        start=True,
                        stop=True,
                        tile_position=(b, 0),
                    )

                filler(2 if p < 8 else 1)
                exp_stage(kA, psA, 0)
                exp_stage(kB, psB, 0)

                while (
                    state["next_red"] * 4 + 3 <= kB - PIPE_D
                    and state["next_red"] * 4 + 3 < NCHUNK
                ):
                    reduce_quad(state["next_red"])
                    state["next_red"] += 1

            while state["next_red"] < NCHUNK // 4:
                reduce_quad(state["next_red"])
                state["next_red"] += 1
    nc.compile()
    return nc


def _host_prep(x, centers, coefficients):
    """O(M) center prep + per-core x layout, all in log2 units."""
    x = np.ascontiguousarray(np.asarray(x, dtype=np.float32))
    centers = np.asarray(centers, dtype=np.float32)
    coefficients = np.asarray(coefficients, dtype=np.float32)

    norm_const = np.float32(1.0 / ((2.0 * math.pi) ** (D / 2) * SIGMA**D))
    e = np.exp(coefficients - coefficients.max())
    w = (e / e.sum()).astype(np.float32)

    s = np.float32(math.sqrt(LOG2E))
    b = centers.T * s  # [4, 256]
    b_hi = b.astype(np.float16)
    b_lo = (b - b_hi.astype(np.float32)).astype(np.float16)

    g_raw = (
        np.log2(w * norm_const) - 0.5 * LOG2E * (centers**2).sum(axis=1)
    ).astype(np.float32)
    S = np.float32(math.floor(12.0 - np.log2(w * norm_const).max()))
    g = g_raw + S
    g_hi = g.astype(np.float16)
    g_lo = (g - g_hi.astype(np.float32)).astype(np.float16)

    halfw = np.zeros((2, 16, 128), dtype=np.float16)
    for h in range(2):
        sl = slice(128 * h, 128 * (h + 1))
        halfw[h, 0:4] = b_hi[:, sl]
        halfw[h, 4:8] = b_hi[:, sl]
        halfw[h, 8:12] = b_lo[:, sl]
        halfw[h, 12] = 1.0
        halfw[h, 13] = 1.0
        halfw[h, 14] = g_hi[sl]
        halfw[h, 15] = g_lo[sl]
    lhsT = np.zeros((128, 128), dtype=np.float16)
    lhsT[0:16] = halfw[0]
    lhsT[32:48] = halfw[1]
    lhsT[64:80] = halfw[0]
    lhsT[96:112] = halfw[1]

    in_maps = []
    for i in range(N_CORES):
        xs = x[i * PER_CORE : (i + 1) * PER_CORE]
        xp = np.zeros((NP, D), dtype=np.float32)
        xp[:PER_CORE] = xs
        a = xp * s
        a_hi = a.astype(np.float16)
        a_lo = (a - a_hi.astype(np.float32)).astype(np.float16)
        hbias = (-0.5 * LOG2E * (xp**2).sum(axis=1)).astype(np.float32)
        h_hi = hbias.astype(np.float16)
        h_lo = (hbias - h_hi.astype(np.float32)).astype(np.float16)
        rows = np.empty((16, NP), dtype=np.float16)
        rows[0:4] = a_hi.T
        rows[4:8] = a_lo.T
        rows[8:12] = a_hi.T
        rows[12] = h_hi
        rows[13] = h_lo
        rows[14] = 1.0
        rows[15] = 1.0
        # pack per group: 64 lines = {A rows, A rows, B rows, B rows},
        # pair columns side by side; groups = (1, 1, GP, GP, ...) pairs
        rc = rows.reshape(16, NCHUNK, CHUNK).transpose(1, 0, 2)  # [124,16,512]
        rp = rc.reshape(NPAIR, 2, 16, CHUNK)
        gs_list = []
        p0 = 0
        for gs in (GP,) * 100:
            if p0 >= NPAIR:
                break
            gs = min(gs, NPAIR - p0)
            gs_list.append((p0, gs))
            p0 += gs
        rhs = np.zeros((NGROUP + 2, 64, GP * CHUNK), dtype=np.float16)
        for gi, (p0, gsz) in enumerate(gs_list):
            for j in range(gsz):
                cs = slice(j * CHUNK, (j + 1) * CHUNK)
                rhs[gi, 0:16, cs] = rp[p0 + j, 0]
                rhs[gi, 16:32, cs] = rp[p0 + j, 0]
                rhs[gi, 32:48, cs] = rp[p0 + j, 1]
                rhs[gi, 48:64, cs] = rp[p0 + j, 1]
        in_maps.append({"rhs": rhs, "lhsT": lhsT.copy()})
    return in_maps, float(S)


last_result = None


def kernel(x, centers, coefficients):
    global last_result
    if "nc" not in _CACHE:
        _CACHE["nc"] = _build_nc()
    nc = _CACHE["nc"]
    in_maps, S = _host_prep(x, centers, coefficients)
    res = run_bass_kernel_spmd(nc, in_maps, core_ids=list(range(N_CORES)))
    last_result = res
    y = np.concatenate([r["y"][:PER_CORE] for r in res.results])
    return (y * np.float32(2.0 ** (-S))).astype(np.float32)


# revision 30
# speedup vs baseline: 1.3427x; 1.0043x over previous
"""Trainium2 Bass kernel for GaussianKernelLayer.

y[n] = sum_m softmax(coef)[m] * norm * exp(-0.5*|x_n - c_m|^2),
N=500000, M=256, D=4, sigma=1. Data-parallel over 8 cores (x sharded on N).

Final design (per core, NP=63488 padded rows, 124 chunks of 512),
~114 us HW vs the 222 us starting point:
  - One K=32 fp16 matmul per (chunk, m-half) assembles the FULL exp
    argument in log2 units directly in PSUM:
      psum[m, n] = log2e*(x.c - 0.5|x|^2) + [log2(w_m*norm)
                   - 0.5*log2e*|c_m|^2 + S]  =: t'   (gauss = 2^t')
    via hi/lo fp16 splits of x*sqrt(log2e) and c*sqrt(log2e) plus bias
    rows, so no separate bias/softmax work on device.
  - Matmuls are issued in QUADS of adjacent instructions targeting the
    4 PE row-groups (mains: 2 chunks x 2 m-halves at partition bands
    0/32/64/96, K padded 16->32 with zero weight rows) or the 4
    col-groups (ones-reduce over m, K=128) -> each quad executes
    concurrently in the 128x128 array (~215 ns warm per quad).
  - The TRN2 HAM clock-gate keeps the PE at 1.2 GHz unless the array
    is near-fully active for ~3.4 us: a 14-matmul full-array warm-up
    flips it to 2.4 GHz and K=128 "filler" matmuls on scratch data at
    quad boundaries (dedicated PSUM bank) absorb would-be idle so it
    never re-throttles. Measured: K=16 streams NEVER warm up; losing
    warmth costs ~1.6x on every matmul.
  - exp splits across two engines, strictly alternating per chunk:
      ACT: activation(Exp, scale=ln2) -> exact 2^t' (fp16)
      DVE: Schraudolph in ONE tensor_scalar: uint16(t'*1024+15*1024-C)
           with saturating float->uint16 convert; the bit pattern IS
           fp16(2^t') up to piecewise-linear mantissa (C=60 tuned,
           contributes ~2e-3 rel L2 vs the 2e-2 budget).
  - Reduce over m: ones-matmul col-group quads with PSUM accumulation
    of the two halves, software-pipelined PIPE_D chunks behind the
    mains so the in-order PE queue never head-of-line stalls. y is
    evacuated PSUM->SBUF on ACT/DVE (balanced) and DMA'd by GPSIMD.
  - rhs is packed on host so ONE DMA per band fills a 64-partition-line
    group (2 chunks x 2 row-group bands x GP pairs); band DMAs split
    across the sync and gpsimd queues. Host does the O(M) prep and the
    final 2^-S rescale.
"""

import math

import numpy as np

import concourse.bass as bass
import concourse.bacc as bacc_mod
import concourse.mybir as mybir
from concourse.bass_utils import run_bass_kernel_spmd
from concourse.tile import TileContext

N_CORES = 8
N_TOTAL = 500000
PER_CORE = N_TOTAL // N_CORES  # 62500
CHUNK = 512
NCHUNK = 124
NP = CHUNK * NCHUNK  # 63488
M = 256
D = 4
SIGMA = 1.0

F16 = mybir.dt.float16
F32 = mybir.dt.float32
U16 = mybir.dt.uint16

LOG2E = 1.0 / math.log(2.0)
LN2 = math.log(2.0)
SCH_C = 60.0  # Schraudolph shift, tuned on host sim
SCH_BIAS = float(15 * 1024 - SCH_C)

# chunk -> exp engine (1 = ACT exact exp, 0 = DVE Schraudolph);
# pure alternating: every pair = (ACT, DVE) for maximal engine overlap;
# the ACT/DVE rate gap is compensated via evac-copy placement
ACT_PATTERN = (1, 0)

PIPE_D = 10  # reduce stage lags the matmul stage by this many chunks
FILL_FD = 384  # keep-warm filler free-dim
EVAC_DVE_MOD = 3  # evac j goes to DVE when j % EVAC_DVE_MOD == 0, else ACT
USE_GPSIMD_DMA = True  # False -> all DMAs on the sync queue
WARMUP_MM = 14
GP = 4  # chunk-PAIRS per rhs DMA group (8 chunks)
NPAIR = NCHUNK // 2  # 62
NGROUP = (NPAIR + GP - 1) // GP  # 16
NRHSBUF = 4

_CACHE = {}


def _build_nc():
    nc = bacc_mod.Bacc()

    # rhs packed per group of GP chunk-pairs: 64 partition-lines = {A-chunk
    # rows, A rows again, B rows, B rows again} x (pair cols side by side).
    # First two groups are single pairs so the exp pipeline starts early.
    rhs_d = nc.dram_tensor("rhs", [NGROUP + 2, 64, GP * CHUNK], F16, kind="ExternalInput")
    # full K=32-padded weights for the four row-group bands (zeros included)
    lhsT_d = nc.dram_tensor("lhsT", [128, 128], F16, kind="ExternalInput")
    y_d = nc.dram_tensor("y", [NP], F32, kind="ExternalOutput")

    with TileContext(nc) as tc:
        with (
            tc.tile_pool(name="const", bufs=1) as constp,
            tc.tile_pool(name="cbp", bufs=PIPE_D + 5) as cbp,
            tc.tile_pool(name="ycp", bufs=3) as ycp,
            tc.tile_pool(name="psp", bufs=3, space="PSUM") as psp,
            tc.tile_pool(name="yp", bufs=1, space="PSUM") as yp,
            tc.tile_pool(name="fillp", bufs=1, space="PSUM") as fillp,
        ):
            # --- constants ---
            lhsT_sb = constp.tile([128, 128], F16)
            nc.sync.dma_start(lhsT_sb[:], lhsT_d[:])
            ones_red = constp.tile([128, 32], F16)
            nc.vector.memset(ones_red[:], 1.0)
            scratch = constp.tile([128, CHUNK], F16)
            nc.vector.memset(scratch[:], 0.0)

            # static rhs buffers; odd 16-line bands are zeroed ONCE (they are
            # read by the K=32-padded matmuls against zero weight rows and
            # must not contain NaN junk)
            rhs_bufs = []
            for i in range(NRHSBUF):
                rb = constp.tile([128, GP * CHUNK], F16, name=f"rhsbuf{i}")
                if i < 2:  # bufs 2-3 are zeroed lazily before first use
                    nc.vector.memset(rb[:], 0.0)
                rhs_bufs.append(rb)
            memset_done = {0, 1}

            # --- HAM warm-up: serial full-array K=128 matmuls on memset
            # data (the one shape measured to flip the clock-gate to 2.4
            # GHz; once warm it stays warm - re-throttle needs ~3.4us of
            # CONTIGUOUS PE idle, which the steady state never has) ---
            ps_w = psp.tile([128, 2 * CHUNK], F32, tag="ps")
            for i in range(WARMUP_MM):
                nc.tensor.matmul(
                    ps_w[:, CHUNK * (i % 2) : CHUNK * (i % 2 + 1)],
                    scratch[:, 0:128],
                    scratch[:, 0:CHUNK],
                    start=True,
                    stop=True,
                )

            # keep-warm filler: full-array K=128 matmuls on scratch into a
            # dedicated PSUM bank. The HAM clock-gate re-throttles the PE to
            # 1.2 GHz whenever array activity drops for a ~3.4us window; the
            # real work (K=32 row-tiles + 32-col reduces) alone doesn't
            # sustain enough activity. Fillers at quad boundaries absorb
            # would-be idle and keep the 2.4 GHz clock.
            fill_ps = fillp.tile([128, CHUNK], F32)

            def filler(n=1, fd=None):
                for _ in range(n):
                    nc.tensor.matmul(
                        fill_ps[:, 0 : (fd or FILL_FD)],
                        scratch[:, 0:128],
                        scratch[:, 0 : (fd or FILL_FD)],
                        start=True,
                        stop=True,
                    )

            cbs = {}  # chunk k -> cb fp16 AP
            state = {"yps": None, "next_red": 0}

            def reduce_quad(j):
                """ones-reduce for chunks 4j..4j+3, quad-concurrent."""
                yps = yp.tile([128, CHUNK], F32, tag="yps", name=f"yps_{j}")
                state["yps"] = yps
                quad = [cbs.pop(4 * j + q) for q in range(4)]
                for h in range(2):  # half 0 then half 1 (accumulate)
                    for q in range(4):
                        nc.tensor.matmul(
                            yps[32 * q : 32 * q + 32, :],
                            ones_red[:],
                            quad[q][:, h * CHUNK : (h + 1) * CHUNK],
                            start=(h == 0),
                            stop=(h == 1),
                            tile_position=(0, 32 * q),
                        )
                filler(2 if j < 8 else 1)
                yc = ycp.tile([128, CHUNK], F32, tag="yc")
                if j % EVAC_DVE_MOD == 0:
                    nc.vector.tensor_copy(yc[:], yps[:])
                else:
                    nc.scalar.copy(yc[:], yps[:])
                yeng = nc.gpsimd if USE_GPSIMD_DMA else nc.sync
                yeng.dma_start(
                    y_d[4 * j * CHUNK : (4 * j + 4) * CHUNK].rearrange(
                        "(p f) -> p f", p=4
                    ),
                    yc[0:97:32, :],
                )

            def exp_stage(k, ps, lo):
                """exp of chunk k from psum tile ps columns [lo, lo+1024)."""
                if ACT_PATTERN[k % len(ACT_PATTERN)]:
                    cb = cbp.tile([128, 2 * CHUNK], F16, tag="cb", name=f"cb_{k}")
                    nc.scalar.activation(
                        cb[:],
                        ps[:, lo : lo + 2 * CHUNK],
                        mybir.ActivationFunctionType.Exp,
                        scale=LN2,
                    )
                    cbs[k] = cb[:]
                else:
                    cb = cbp.tile([128, 2 * CHUNK], U16, tag="cb", name=f"cb_{k}")
                    nc.vector.tensor_scalar(
                        cb[:],
                        ps[:, lo : lo + 2 * CHUNK],
                        1024.0,
                        SCH_BIAS,
                        mybir.AluOpType.mult,
                        mybir.AluOpType.add,
                    )
                    cbs[k] = cb[:].bitcast(F16)

            # group schedule (in pairs): two single-pair ramp-up groups,
            # then full GP-pair groups
            group_start = []
            group_sz = []
            p0 = 0
            for gs in (GP,) * 100:
                if p0 >= NPAIR:
                    break
                gs = min(gs, NPAIR - p0)
                group_start.append(p0)
                group_sz.append(gs)
                p0 += gs
            group_of = []
            for gi2, gs in enumerate(group_sz):
                group_of.extend([gi2] * gs)

            for p in range(NPAIR):
                gi = group_of[p]
                jj = p - group_start[gi]
                if jj == 0:
                    gsz = group_sz[gi]
                    rhs_t = rhs_bufs[gi % NRHSBUF]
                    if gi % NRHSBUF not in memset_done:
                        memset_done.add(gi % NRHSBUF)
                        nc.vector.memset(rhs_t[:], 0.0)
                    eng2 = nc.gpsimd if USE_GPSIMD_DMA else nc.sync
                    for b, eng in (
                        (0, nc.sync),
                        (1, eng2),
                        (2, nc.sync),
                        (3, eng2),
                    ):
                        eng.dma_start(
                            rhs_t[32 * b : 32 * b + 16, 0 : gsz * CHUNK],
                            rhs_d[gi, 16 * b : 16 * b + 16, 0 : gsz * CHUNK],
                        )
                col = jj * CHUNK
                kA, kB = 2 * p, 2 * p + 1
                psA = psp.tile([128, 2 * CHUNK], F32, tag="ps", name=f"psA_{p}")
                psB = psp.tile([128, 2 * CHUNK], F32, tag="ps", name=f"psB_{p}")
                # quad: (A,h0)->band0, (A,h1)->band32, (B,h0)->band64,
                # (B,h1)->band96 -- concurrent row-group tiles
                for b, ps, lo in (
                    (0, psA, 0),
                    (32, psA, CHUNK),
                    (64, psB, 0),
                    (96, psB, CHUNK),
                ):
                    nc.tensor.matmul(
                        ps[:, lo : lo + CHUNK],
                        lhsT_sb[b : b + 32, :],
                        rhs_t[b : b + 32, col : col + CHUNK],
                        start=True,
                        stop=True,
                        tile_position=(b, 0),
                    )

                filler(2 if p < 8 else 1)
                exp_stage(kA, psA, 0)
                exp_stage(kB, psB, 0)

                while (
                    state["next_red"] * 4 + 3 <= kB - PIPE_D
                    and state["next_red"] * 4 + 3 < NCHUNK
                ):
                    reduce_quad(state["next_red"])
                    state["next_red"] += 1

            while state["next_red"] < NCHUNK // 4:
                reduce_quad(state["next_red"])
                state["next_red"] += 1
    nc.compile()
    return nc


def _host_prep(x, centers, coefficients):
    """O(M) center prep + per-core x layout, all in log2 units."""
    x = np.ascontiguousarray(np.asarray(x, dtype=np.float32))
    centers = np.asarray(centers, dtype=np.float32)
    coefficients = np.asarray(coefficients, dtype=np.float32)

    norm_const = np.float32(1.0 / ((2.0 * math.pi) ** (D / 2) * SIGMA**D))
    e = np.exp(coefficients - coefficients.max())
    w = (e / e.sum()).astype(np.float32)

    s = np.float32(math.sqrt(LOG2E))
    b = centers.T * s  # [4, 256]
    b_hi = b.astype(np.float16)
    b_lo = (b - b_hi.astype(np.float32)).astype(np.float16)

    g_raw = (
        np.log2(w * norm_const) - 0.5 * LOG2E * (centers**2).sum(axis=1)
    ).astype(np.float32)
    S = np.float32(math.floor(12.0 - np.log2(w * norm_const).max()))
    g = g_raw + S
    g_hi = g.astype(np.float16)
    g_lo = (g - g_hi.astype(np.float32)).astype(np.float16)

    halfw = np.zeros((2, 16, 128), dtype=np.float16)
    for h in range(2):
        sl = slice(128 * h, 128 * (h + 1))
        halfw[h, 0:4] = b_hi[:, sl]
        halfw[h, 4:8] = b_hi[:, sl]
        halfw[h, 8:12] = b_lo[:, sl]
        halfw[h, 12] = 1.0
        halfw[h, 13] = 1.0
        halfw[h, 14] = g_hi[sl]
        halfw[h, 15] = g_lo[sl]
    lhsT = np.zeros((128, 128), dtype=np.float16)
    lhsT[0:16] = halfw[0]
    lhsT[32:48] = halfw[1]
    lhsT[64:80] = halfw[0]
    lhsT[96:112] = halfw[1]

    in_maps = []
    for i in range(N_CORES):
        xs = x[i * PER_CORE : (i + 1) * PER_CORE]
        xp = np.zeros((NP, D), dtype=np.float32)
        xp[:PER_CORE] = xs
        a = xp * s
        a_hi = a.astype(np.float16)
        a_lo = (a - a_hi.astype(np.float32)).astype(np.float16)
        hbias = (-0.5 * LOG2E * (xp**2).sum(axis=1)).astype(np.float32)
        h_hi = hbias.astype(np.float16)
        h_lo = (hbias - h_hi.astype(np.float32)).astype(np.float16)
        rows = np.empty((16, NP), dtype=np.float16)
        rows[0:4] = a_hi.T
        rows[4:8] = a_lo.T
        rows[8:12] = a_hi.T
        rows[12] = h_hi
        rows[13] = h_lo
        rows[14] = 1.0
        rows[15] = 1.0
        # pack per group: 64 lines = {A rows, A rows, B rows, B rows},
        # pair columns side by side; groups = (1, 1, GP, GP, ...) pairs
        rc = rows.reshape(16, NCHUNK, CHUNK).transpose(1, 0, 2)  # [124,16,512]
        rp = rc.reshape(NPAIR, 2, 16, CHUNK)
        gs_list = []
        p0 = 0
        for gs in (GP,) * 100:
            if p0 >= NPAIR:
                break
            gs = min(gs, NPAIR - p0)
            gs_list.append((p0, gs))
            p0 += gs
        rhs = np.zeros((NGROUP + 2, 64, GP * CHUNK), dtype=np.float16)
        for gi, (p0, gsz) in enumerate(gs_list):
            for j in range(gsz):
                cs = slice(j * CHUNK, (j + 1) * CHUNK)
                rhs[gi, 0:16, cs] = rp[p0 + j, 0]
                rhs[gi, 16:32, cs] = rp[p0 + j, 0]
                rhs[gi, 32:48, cs] = rp[p0 + j, 1]
                rhs[gi, 48:64, cs] = rp[p0 + j, 1]
        in_maps.append({"rhs": rhs, "lhsT": lhsT.copy()})
    return in_maps, float(S)


last_result = None


def kernel(x, centers, coefficients):
    global last_result
    if "nc" not in _CACHE:
        _CACHE["nc"] = _build_nc()
    nc = _CACHE["nc"]
    in_maps, S = _host_prep(x, centers, coefficients)
    res = run_bass_kernel_spmd(nc, in_maps, core_ids=list(range(N_CORES)))
    last_result = res
    y = np.concatenate([r["y"][:PER_CORE] for r in res.results])
    return (y * np.float32(2.0 ** (-S))).astype(np.float32)
